# revision 8
# baseline (speedup 1.0000x reference)
"""GegenbauerKAN layer (alpha=1 -> Chebyshev-U basis) on 8 TRN2 NeuronCores.

Math: y[b,o] = sum_{i,d} C_d(tanh(x[b,i])) * W[i,o,d],  d=0..7,
where C_d are Gegenbauer(alpha=1) = Chebyshev-U polynomials.

Strategy:
  - Data-parallel over batch: each of the 8 cores handles 2048 rows.
  - Transposed layout: the host feeds x^T slices so the contraction
    index i lives on SBUF partitions with no on-device transposes.
  - On-device basis: exact U_d values via Chebyshev addition formulas
    (U_{m+n} = U_m U_n - U_{m-1} U_{n-1}), all in fp32:
        t  = tanh(x)            s4 = (2t)^2 = U2+1
        b3 = (s4-2)t = U3/2     q2 = (s4-1)^2 = U2^2
        b4 = q2-s4   = U4       d  = b3-t    = (U3-U1)/2
        b5 = (s4-1)d = U5/2     q3 = b3^2    = U3^2/4
        b6 = 4q3-q2  = U6       e  = b4+1-s4 = U4-U2
        b7 = e*b3    = U7/2
    This is numerically identical in conditioning to the textbook
    recurrence but needs only 7 DVE + 4 ACT ops per chunk.
  - Matmuls run in float32r (TRN2 fast fp32 PE path, 1 cycle/row for
    moving dim >= 256, ~= 11-bit-mantissa rounding of the operands).
    Basis tensors and weights are rounded to f32r exactly once, at the
    matmul boundary; all basis arithmetic stays fp32. Measured/simulated
    end-to-end max error ~1.7e-4 * absmax(y) (fp32 ref itself: ~5e-6).
  - k=0 (U_0 = 1) is folded into a per-output bias computed on-device
    from the k=0 weight block with ones-vector matmuls, added at PSUM
    eviction (saves 1/8 of the matmul work).
  - Weights basis change on host: y = sum_k phi_k . V_k with
    V[:,:,k] = sum_d W[:,:,d] M[d,k], M the (exact, tiny) change of
    basis from {phi_k} to {U_d}; applied in fp64, rounded once.
"""

import numpy as np

import concourse.bacc as bacc
import concourse.mybir as mybir
import concourse.tile as tile
from concourse.alu_op_type import AluOpType as ALU
from concourse.bass_utils import run_bass_kernel_spmd

F32 = mybir.dt.float32
F32R = mybir.dt.float32r
AF = mybir.ActivationFunctionType

N_CORES = 8
B = 16384
I = 512
O = 512
DEG = 8  # degrees 0..7
B_LOC = B // N_CORES  # 2048 rows per core
CHUNK = 512  # b columns processed per pipeline stage
N_CHUNKS = B_LOC // CHUNK
IT = I // 128  # 4 partition tiles of the input-feature dim
OT = O // 128  # 4 partition tiles of the output dim


def _basis_matrix() -> np.ndarray:
    """M[d,k]: U_d = sum_k M[d,k] * phi_k for the on-device basis
    phi = [1, t, s4, b3, b4, b5, b6, b7]."""
    M = np.zeros((DEG, DEG))
    M[0, 0] = 1.0
    M[1, 1] = 2.0  # U1 = 2 t
    M[2, 0] = -1.0
    M[2, 2] = 1.0  # U2 = s4 - 1
    M[3, 3] = 2.0  # U3 = 2 b3
    M[4, 4] = 1.0  # U4 = b4
    M[5, 5] = 2.0  # U5 = 2 b5
    M[6, 6] = 1.0  # U6 = b6
    M[7, 7] = 2.0  # U7 = 2 b7
    return M


def _build_nc():
    nc = bacc.Bacc("TRN2", target_bir_lowering=False, debug=False)

    xt = nc.dram_tensor("xt", [I, B_LOC], F32, kind="ExternalInput")
    wv = nc.dram_tensor("wv", [DEG * I, O], F32, kind="ExternalInput")
    yt = nc.dram_tensor("yt", [O, B_LOC], F32, kind="ExternalOutput")

    with tile.TileContext(nc) as tc:
        with (
            tc.tile_pool(name="wvp", bufs=1) as wvp,
            tc.tile_pool(name="sb", bufs=1) as sb,
            tc.tile_pool(name="xtp", bufs=1) as xtp,
            tc.tile_pool(name="outp", bufs=2) as outp,
            tc.tile_pool(name="ps", bufs=4, space="PSUM") as ps,
            tc.tile_pool(name="bps", bufs=2, space="PSUM") as bps,
        ):
            # Weights: 8 tiles [128, IT, 512] (k-major rows), cast to f32r
            # by the gpsimd DMA.
            wv_sb = []
            for k in range(DEG):
                w = wvp.tile([128, IT, O], F32R, tag=f"wv{k}")
                nc.gpsimd.dma_start(
                    out=w[:],
                    in_=wv[k * I : (k + 1) * I, :].rearrange(
                        "(a p) o -> p a o", p=128
                    ),
                )
                wv_sb.append(w)

            # Per-output bias from the k=0 block: bias[o] = sum_i V[i,o,0].
            ones_f = sb.tile([128, 8], F32, tag="ones_f")
            nc.vector.memset(ones_f[:], 1.0)
            ones = sb.tile([128, 8], F32R, tag="ones")
            nc.vector.tensor_copy(ones[:], ones_f[:])
            bias_sb = sb.tile([128, OT], F32, tag="bias")
            neg1 = sb.tile([128, 1], F32, tag="neg1")
            nc.vector.memset(neg1[:], -1.0)
            for j in range(OT):
                bp = bps.tile([128, 8], F32, tag="bps")
                for a in range(IT):
                    nc.tensor.matmul(
                        bp[:],
                        lhsT=wv_sb[0][:, a, j * 128 : (j + 1) * 128],
                        rhs=ones[:],
                        start=(a == 0),
                        stop=(a == IT - 1),
                    )
                nc.scalar.activation(bias_sb[:, j : j + 1], bp[:, 0:1], AF.Copy)

            flat = [128, IT * CHUNK]
            for c in range(N_CHUNKS):
                bsl = slice(c * CHUNK, (c + 1) * CHUNK)
                x_sb = xtp.tile([128, IT, CHUNK], F32, tag="xt")
                nc.sync.dma_start(
                    out=x_sb[:],
                    in_=xt[:, bsl].rearrange("(a p) b -> p a b", p=128),
                )
                xf = x_sb[:].rearrange("p a b -> p (a b)")

                # --- fp32 basis DAG (exact U_d up to scale) ---
                t = sb.tile(flat, F32, tag="t")
                nc.scalar.activation(t[:], xf, AF.Tanh)
                s4 = sb.tile(flat, F32, tag="s4")
                nc.scalar.activation(s4[:], t[:], AF.Square, scale=2.0)
                b3 = sb.tile(flat, F32, tag="b3")
                nc.vector.scalar_tensor_tensor(
                    b3[:], s4[:], 2.0, t[:], ALU.subtract, ALU.mult
                )
                q2 = sb.tile(flat, F32, tag="q2")
                nc.scalar.activation(q2[:], s4[:], AF.Square, bias=neg1[:])
                b4 = sb.tile(flat, F32, tag="b4")
                nc.vector.tensor_sub(b4[:], q2[:], s4[:])
                d = sb.tile(flat, F32, tag="de")
                nc.vector.tensor_sub(d[:], b3[:], t[:])
                b5 = sb.tile(flat, F32R, tag="b5")
                nc.vector.scalar_tensor_tensor(
                    b5[:], s4[:], 1.0, d[:], ALU.subtract, ALU.mult
                )
                q3 = sb.tile(flat, F32, tag="q3")
                nc.scalar.activation(q3[:], b3[:], AF.Square)
                b6 = sb.tile(flat, F32R, tag="b6")
                nc.vector.scalar_tensor_tensor(
                    b6[:], q3[:], 4.0, q2[:], ALU.mult, ALU.subtract
                )
                e = sb.tile(flat, F32, tag="de")
                nc.vector.scalar_tensor_tensor(
                    e[:], b4[:], 1.0, s4[:], ALU.add, ALU.subtract
                )
                b7 = sb.tile(flat, F32R, tag="b7")
                nc.vector.tensor_mul(b7[:], e[:], b3[:])

                # f32r boundary copies for reused fp32 tensors
                t_r = sb.tile(flat, F32R, tag="t_r")
                nc.gpsimd.tensor_copy(t_r[:], t[:])
                s4_r = sb.tile(flat, F32R, tag="s4_r")
                nc.gpsimd.tensor_copy(s4_r[:], s4[:])
                b3_r = sb.tile(flat, F32R, tag="b3_r")
                nc.gpsimd.tensor_copy(b3_r[:], b3[:])
                b4_r = sb.tile(flat, F32R, tag="b4_r")
                nc.gpsimd.tensor_copy(b4_r[:], b4[:])

                basis = [t_r, s4_r, b3_r, b4_r, b5, b6, b7]  # k = 1..7

                for j in range(OT):
                    acc = ps.tile([128, CHUNK], F32, tag="acc")
                    n_mm = (DEG - 1) * IT
                    idx = 0
                    for k in range(1, DEG):
                        pk = basis[k - 1]
                        for a in range(IT):
                            nc.tensor.matmul(
                                acc[:],
                                lhsT=wv_sb[k][:, a, j * 128 : (j + 1) * 128],
                                rhs=pk[:, a * CHUNK : (a + 1) * CHUNK],
                                start=(idx == 0),
                                stop=(idx == n_mm - 1),
                            )
                            idx += 1
                    o_sb = outp.tile([128, CHUNK], F32, tag="out")
                    nc.scalar.activation(
                        o_sb[:], acc[:], AF.Identity, bias=bias_sb[:, j : j + 1]
                    )
                    nc.sync.dma_start(
                        out=yt[j * 128 : (j + 1) * 128, bsl], in_=o_sb[:]
                    )

    nc.compile()
    return nc


_NC_CACHE = None
_last_in_maps = None


def _get_nc():
    global _NC_CACHE
    if _NC_CACHE is None:
        _NC_CACHE = _build_nc()
    return _NC_CACHE


def kernel(x: np.ndarray, gegenbauer_coeffs: np.ndarray, **unused) -> np.ndarray:
    x = np.asarray(x, dtype=np.float32).reshape(B, I)
    coeffs = np.asarray(gegenbauer_coeffs, dtype=np.float32)

    # Host prep: basis change (exact integers, applied in fp64) and layouts.
    M = _basis_matrix()
    v = np.einsum("iod,dk->kio", coeffs.astype(np.float64), M)
    wv = np.ascontiguousarray(v.reshape(DEG * I, O).astype(np.float32))
    xt_full = np.ascontiguousarray(x.T)  # [I, B]

    in_maps = []
    for c in range(N_CORES):
        xt_c = np.ascontiguousarray(xt_full[:, c * B_LOC : (c + 1) * B_LOC])
        in_maps.append({"xt": xt_c, "wv": wv})

    global _last_in_maps
    _last_in_maps = in_maps

    nc = _get_nc()
    try:
        res = run_bass_kernel_spmd(nc, in_maps, core_ids=list(range(N_CORES)))
    except Exception:
        # A previous crashed session can leave a core unrecoverable until
        # the runtime resets it; one retry clears it.
        res = run_bass_kernel_spmd(nc, in_maps, core_ids=list(range(N_CORES)))

    y = np.empty((B, O), dtype=np.float32)
    for c in range(N_CORES):
        y[c * B_LOC : (c + 1) * B_LOC, :] = res.results[c]["yt"].T
    return y


# revision 11
# speedup vs baseline: 1.6596x; 1.6596x over previous
"""GegenbauerKAN layer (alpha=1 -> Chebyshev-U basis) on 8 TRN2 NeuronCores.

Math: y[b,o] = sum_{i,d} C_d(tanh(x[b,i])) * W[i,o,d],  d=0..7,
where C_d are Gegenbauer(alpha=1) = Chebyshev-U polynomials.

Strategy:
  - Data-parallel over batch: each of the 8 cores handles 2048 rows.
  - Transposed layout: the host feeds x^T slices so the contraction
    index i lives on SBUF partitions with no on-device transposes.
  - On-device basis: exact U_d values via Chebyshev addition formulas
    (U_{m+n} = U_m U_n - U_{m-1} U_{n-1}), all in fp32:
        t  = tanh(x)            s4 = (2t)^2 = U2+1
        b3 = (s4-2)t = U3/2     q2 = (s4-1)^2 = U2^2
        b4 = q2-s4   = U4       d  = b3-t    = (U3-U1)/2
        b5 = (s4-1)d = U5/2     q3 = b3^2    = U3^2/4
        b6 = 4q3-q2  = U6       e  = b4+1-s4 = U4-U2
        b7 = e*b3    = U7/2
    This is numerically identical in conditioning to the textbook
    recurrence but needs only 7 DVE + 4 ACT ops per chunk.
  - Matmuls run in float32r (TRN2 fast fp32 PE path, 1 cycle/row for
    moving dim >= 256, ~= 11-bit-mantissa rounding of the operands).
    Basis tensors and weights are rounded to f32r exactly once, at the
    matmul boundary; all basis arithmetic stays fp32. Measured/simulated
    end-to-end max error ~1.7e-4 * absmax(y) (fp32 ref itself: ~5e-6).
  - k=0 (U_0 = 1) is folded into a per-output bias computed on-device
    from the k=0 weight block with ones-vector matmuls, added at PSUM
    eviction (saves 1/8 of the matmul work).
  - Weights basis change on host: y = sum_k phi_k . V_k with
    V[:,:,k] = sum_d W[:,:,d] M[d,k], M the (exact, tiny) change of
    basis from {phi_k} to {U_d}; applied in fp64, rounded once.
"""

import numpy as np

import concourse.bacc as bacc
import concourse.mybir as mybir
import concourse.tile as tile
from concourse.alu_op_type import AluOpType as ALU
from concourse.bass_utils import run_bass_kernel_spmd

F32 = mybir.dt.float32
F32R = mybir.dt.float32r
AF = mybir.ActivationFunctionType

N_CORES = 8
B = 16384
I = 512
O = 512
DEG = 8  # degrees 0..7
B_LOC = B // N_CORES  # 2048 rows per core
CHUNK = 512  # b columns processed per pipeline stage
N_CHUNKS = B_LOC // CHUNK
IT = I // 128  # 4 partition tiles of the input-feature dim
OT = O // 128  # 4 partition tiles of the output dim


def _basis_matrix() -> np.ndarray:
    """M[d,k]: U_d = sum_k M[d,k] * phi_k for the on-device basis
    phi = [1, t, s4, b3, b4, b5, b6, b7]."""
    M = np.zeros((DEG, DEG))
    M[0, 0] = 1.0
    M[1, 1] = 2.0  # U1 = 2 t
    M[2, 0] = -1.0
    M[2, 2] = 1.0  # U2 = s4 - 1
    M[3, 3] = 2.0  # U3 = 2 b3
    M[4, 4] = 1.0  # U4 = b4
    M[5, 5] = 2.0
    M[5, 1] = -2.0  # U5 = 2 b5 - 2 t
    M[6, 6] = 1.0  # U6 = b6
    M[7, 7] = 2.0  # U7 = 2 b7
    return M


def _build_nc():
    nc = bacc.Bacc("TRN2", target_bir_lowering=False, debug=False)

    xt = nc.dram_tensor("xt", [I, B_LOC], F32, kind="ExternalInput")
    wv = nc.dram_tensor("wv", [DEG * I, O], F32, kind="ExternalInput")
    yt = nc.dram_tensor("yt", [O, B_LOC], F32, kind="ExternalOutput")

    with tile.TileContext(nc) as tc:
        with (
            tc.tile_pool(name="wvp", bufs=1) as wvp,
            tc.tile_pool(name="sb", bufs=1) as sb,
            tc.tile_pool(name="xtp", bufs=1) as xtp,
            tc.tile_pool(name="outp", bufs=2) as outp,
            tc.tile_pool(name="ps", bufs=4, space="PSUM") as ps,
            tc.tile_pool(name="bps", bufs=2, space="PSUM") as bps,
        ):
            # Weights: 8 tiles [128, IT, 512] (k-major rows), cast to f32r
            # by the gpsimd DMA.
            wv_sb = []
            for k in range(DEG):
                w = wvp.tile([128, IT, O], F32R, tag=f"wv{k}")
                nc.gpsimd.dma_start(
                    out=w[:],
                    in_=wv[k * I : (k + 1) * I, :].rearrange(
                        "(a p) o -> p a o", p=128
                    ),
                )
                wv_sb.append(w)

            # Per-output bias from the k=0 block: bias[o] = sum_i V[i,o,0].
            ones_f = sb.tile([128, 8], F32, tag="ones_f")
            nc.vector.memset(ones_f[:], 1.0)
            ones = sb.tile([128, 8], F32R, tag="ones")
            nc.vector.tensor_copy(ones[:], ones_f[:])
            bias_sb = sb.tile([128, OT], F32, tag="bias")
            neg1 = sb.tile([128, 1], F32, tag="neg1")
            nc.vector.memset(neg1[:], -1.0)
            neg2 = sb.tile([128, 1], F32, tag="neg2")
            nc.vector.memset(neg2[:], -2.0)
            for j in range(OT):
                bp = bps.tile([128, 8], F32, tag="bps")
                for a in range(IT):
                    nc.tensor.matmul(
                        bp[:],
                        lhsT=wv_sb[0][:, a, j * 128 : (j + 1) * 128],
                        rhs=ones[:],
                        start=(a == 0),
                        stop=(a == IT - 1),
                    )
                nc.scalar.activation(bias_sb[:, j : j + 1], bp[:, 0:1], AF.Copy)

            flat = [128, IT * CHUNK]
            for c in range(N_CHUNKS):
                bsl = slice(c * CHUNK, (c + 1) * CHUNK)
                x_sb = xtp.tile([128, IT, CHUNK], F32, tag="xt")
                nc.sync.dma_start(
                    out=x_sb[:],
                    in_=xt[:, bsl].rearrange("(a p) b -> p a b", p=128),
                )
                xf = x_sb[:].rearrange("p a b -> p (a b)")

                # --- fp32 basis DAG (exact U_d up to scale/shift) ---
                # t = tanh x;  s4 = 4t^2 = U2+1;  b3 = (s4-2)t = U3/2
                # q2 = (s4-1)^2 = U2^2;  b4 = q2-s4 = U4
                # b5 = (s4-2)b3 = (U5+2t)/2;  q3 = b3^2
                # b6 = 4q3-q2 = U6;  e4 = (s4-2)^2;  b7 = (e4-2)b3 = U7/2
                t = sb.tile(flat, F32, tag="t")
                nc.scalar.activation(t[:], xf, AF.Tanh)
                s4 = sb.tile(flat, F32, tag="s4")
                nc.scalar.activation(s4[:], t[:], AF.Square, scale=2.0)
                b3 = sb.tile(flat, F32, tag="b3")
                nc.vector.scalar_tensor_tensor(
                    b3[:], s4[:], 2.0, t[:], ALU.subtract, ALU.mult
                )
                q2 = sb.tile(flat, F32, tag="q2")
                nc.scalar.activation(q2[:], s4[:], AF.Square, bias=neg1[:])
                b4 = sb.tile(flat, F32, tag="b4")
                nc.vector.tensor_sub(b4[:], q2[:], s4[:])
                b5 = sb.tile(flat, F32R, tag="b5")
                nc.vector.scalar_tensor_tensor(
                    b5[:], s4[:], 2.0, b3[:], ALU.subtract, ALU.mult
                )
                q3 = sb.tile(flat, F32, tag="q3")
                nc.scalar.activation(q3[:], b3[:], AF.Square)
                b6 = sb.tile(flat, F32R, tag="b6")
                nc.vector.scalar_tensor_tensor(
                    b6[:], q3[:], 4.0, q2[:], ALU.mult, ALU.subtract
                )
                e4 = sb.tile(flat, F32, tag="e4")
                nc.scalar.activation(e4[:], s4[:], AF.Square, bias=neg2[:])
                b7 = sb.tile(flat, F32R, tag="b7")
                nc.vector.scalar_tensor_tensor(
                    b7[:], e4[:], 2.0, b3[:], ALU.subtract, ALU.mult
                )

                # f32r boundary copies for reused fp32 tensors (DVE 2x mode)
                t_r = sb.tile(flat, F32R, tag="t_r")
                nc.vector.tensor_copy(t_r[:], t[:])
                s4_r = sb.tile(flat, F32R, tag="s4_r")
                nc.vector.tensor_copy(s4_r[:], s4[:])
                b3_r = sb.tile(flat, F32R, tag="b3_r")
                nc.vector.tensor_copy(b3_r[:], b3[:])
                b4_r = sb.tile(flat, F32R, tag="b4_r")
                nc.vector.tensor_copy(b4_r[:], b4[:])

                basis = [t_r, s4_r, b3_r, b4_r, b5, b6, b7]  # k = 1..7

                for j in range(OT):
                    acc = ps.tile([128, CHUNK], F32, tag="acc")
                    n_mm = (DEG - 1) * IT
                    idx = 0
                    for k in range(1, DEG):
                        pk = basis[k - 1]
                        for a in range(IT):
                            nc.tensor.matmul(
                                acc[:],
                                lhsT=wv_sb[k][:, a, j * 128 : (j + 1) * 128],
                                rhs=pk[:, a * CHUNK : (a + 1) * CHUNK],
                                start=(idx == 0),
                                stop=(idx == n_mm - 1),
                            )
                            idx += 1
                    o_sb = outp.tile([128, CHUNK], F32, tag="out")
                    nc.scalar.activation(
                        o_sb[:], acc[:], AF.Identity, bias=bias_sb[:, j : j + 1]
                    )
                    nc.sync.dma_start(
                        out=yt[j * 128 : (j + 1) * 128, bsl], in_=o_sb[:]
                    )

    nc.compile()
    return nc


_NC_CACHE = None
_last_in_maps = None


def _get_nc():
    global _NC_CACHE
    if _NC_CACHE is None:
        _NC_CACHE = _build_nc()
    return _NC_CACHE


def kernel(x: np.ndarray, gegenbauer_coeffs: np.ndarray, **unused) -> np.ndarray:
    x = np.asarray(x, dtype=np.float32).reshape(B, I)
    coeffs = np.asarray(gegenbauer_coeffs, dtype=np.float32)

    # Host prep: basis change (exact integers, applied in fp64) and layouts.
    M = _basis_matrix()
    v = np.einsum("iod,dk->kio", coeffs.astype(np.float64), M)
    wv = np.ascontiguousarray(v.reshape(DEG * I, O).astype(np.float32))
    xt_full = np.ascontiguousarray(x.T)  # [I, B]

    in_maps = []
    for c in range(N_CORES):
        xt_c = np.ascontiguousarray(xt_full[:, c * B_LOC : (c + 1) * B_LOC])
        in_maps.append({"xt": xt_c, "wv": wv})

    global _last_in_maps
    _last_in_maps = in_maps

    nc = _get_nc()
    try:
        res = run_bass_kernel_spmd(nc, in_maps, core_ids=list(range(N_CORES)))
    except Exception:
        # A previous crashed session can leave a core unrecoverable until
        # the runtime resets it; one retry clears it.
        res = run_bass_kernel_spmd(nc, in_maps, core_ids=list(range(N_CORES)))

    y = np.empty((B, O), dtype=np.float32)
    for c in range(N_CORES):
        y[c * B_LOC : (c + 1) * B_LOC, :] = res.results[c]["yt"].T
    return y


# revision 20
# speedup vs baseline: 1.6994x; 1.0240x over previous
"""GegenbauerKAN layer (alpha=1 -> Chebyshev-U basis) on 8 TRN2 NeuronCores.

Math: y[b,o] = sum_{i,d} C_d(tanh(x[b,i])) * W[i,o,d],  d=0..7,
where C_d are Gegenbauer(alpha=1) = Chebyshev-U polynomials.

Strategy:
  - Data-parallel over batch: each of the 8 cores handles 2048 rows.
  - Transposed layout: the host feeds x^T slices so the contraction
    index i lives on SBUF partitions with no on-device transposes.
  - On-device basis: exact U_d values via Chebyshev addition formulas
    (U_{m+n} = U_m U_n - U_{m-1} U_{n-1}), all in fp32:
        t  = tanh(x)            s4 = (2t)^2 = U2+1
        b3 = (s4-2)t = U3/2     q2 = (s4-1)^2 = U2^2
        b4 = q2-s4   = U4       d  = b3-t    = (U3-U1)/2
        b5 = (s4-1)d = U5/2     q3 = b3^2    = U3^2/4
        b6 = 4q3-q2  = U6       e  = b4+1-s4 = U4-U2
        b7 = e*b3    = U7/2
    This is numerically identical in conditioning to the textbook
    recurrence but needs only 7 DVE + 4 ACT ops per chunk.
  - Matmuls run in float32r (TRN2 fast fp32 PE path, 1 cycle/row for
    moving dim >= 256, ~= 11-bit-mantissa rounding of the operands).
    Basis tensors and weights are rounded to f32r exactly once, at the
    matmul boundary; all basis arithmetic stays fp32. Measured/simulated
    end-to-end max error ~1.7e-4 * absmax(y) (fp32 ref itself: ~5e-6).
  - k=0 (U_0 = 1) is folded into a per-output bias computed on-device
    from the k=0 weight block with ones-vector matmuls, added at PSUM
    eviction (saves 1/8 of the matmul work).
  - Weights basis change on host: y = sum_k phi_k . V_k with
    V[:,:,k] = sum_d W[:,:,d] M[d,k], M the (exact, tiny) change of
    basis from {phi_k} to {U_d}; applied in fp64, rounded once.
"""

import numpy as np

import concourse.bacc as bacc
import concourse.mybir as mybir
import concourse.tile as tile
from concourse.alu_op_type import AluOpType as ALU
from concourse.bass_utils import run_bass_kernel_spmd

F32 = mybir.dt.float32
F32R = mybir.dt.float32r
AF = mybir.ActivationFunctionType

N_CORES = 8
B = 16384
I = 512
O = 512
DEG = 8  # degrees 0..7
B_LOC = B // N_CORES  # 2048 rows per core
CHUNK = 512  # b columns processed per pipeline stage
N_CHUNKS = B_LOC // CHUNK
IT = I // 128  # 4 partition tiles of the input-feature dim
OT = O // 128  # 4 partition tiles of the output dim


def _basis_matrix() -> np.ndarray:
    """M[d,k]: U_d = sum_k M[d,k] * phi_k for the on-device basis
    phi = [1, t, s4, b3, b4, b5, b6, b7]."""
    M = np.zeros((DEG, DEG))
    M[0, 0] = 1.0
    M[1, 1] = 2.0  # U1 = 2 t
    M[2, 0] = -1.0
    M[2, 2] = 1.0  # U2 = s4 - 1
    M[3, 3] = 2.0  # U3 = 2 b3
    M[4, 4] = 1.0  # U4 = b4
    M[5, 5] = 2.0
    M[5, 1] = -2.0  # U5 = 2 b5 - 2 t
    M[6, 6] = 1.0  # U6 = b6
    M[7, 7] = 2.0  # U7 = 2 b7
    return M


def _build_nc():
    nc = bacc.Bacc("TRN2", target_bir_lowering=False, debug=False)

    xt = nc.dram_tensor("xt", [I, B_LOC], F32, kind="ExternalInput")
    # f32r end-to-end: the PE rounds matmul operands to f32r precision
    # regardless; typing the DRAM tensor f32r lets the fast HWDGE (sync)
    # engine load it with no cast.
    wv = nc.dram_tensor("wv", [DEG * I, O], F32R, kind="ExternalInput")
    yt = nc.dram_tensor("yt", [O, B_LOC], F32, kind="ExternalOutput")

    with tile.TileContext(nc) as tc:
        with (
            tc.tile_pool(name="wvp", bufs=1) as wvp,
            tc.tile_pool(name="sb", bufs=1) as sb,
            tc.tile_pool(name="xtp", bufs=1) as xtp,
            tc.tile_pool(name="outp", bufs=2) as outp,
            tc.tile_pool(name="ps", bufs=6, space="PSUM") as ps,
            tc.tile_pool(name="bps", bufs=2, space="PSUM") as bps,
        ):
            # Chunk 0's x slice first on the sync queue so tanh can start
            # as early as possible.
            x0_sb = xtp.tile([128, IT, CHUNK], F32, tag="xt")
            nc.sync.dma_start(
                out=x0_sb[:],
                in_=xt[:, 0:CHUNK].rearrange("(a p) b -> p a b", p=128),
            )

            # Weights: 8 tiles [128, IT, 512] (k-major rows), split across
            # the gpsimd and sync DMA queues, ordered by when the k-major
            # matmul loop first needs them.
            wv_sb = [None] * DEG
            for qi, k in enumerate([1, 3, 5, 7, 2, 4, 6, 0]):
                w = wvp.tile([128, IT, O], F32R, tag=f"wv{k}")
                eng = nc.gpsimd if qi < 4 else nc.sync
                eng.dma_start(
                    out=w[:],
                    in_=wv[k * I : (k + 1) * I, :].rearrange(
                        "(a p) o -> p a o", p=128
                    ),
                )
                wv_sb[k] = w

            # Per-output bias from the k=0 block: bias[o] = sum_i V[i,o,0].
            ones_f = sb.tile([128, 8], F32, tag="ones_f")
            nc.vector.memset(ones_f[:], 1.0)
            ones = sb.tile([128, 8], F32R, tag="ones")
            nc.vector.tensor_copy(ones[:], ones_f[:])
            bias_sb = sb.tile([128, OT], F32, tag="bias")
            neg1 = sb.tile([128, 1], F32, tag="neg1")
            nc.vector.memset(neg1[:], -1.0)
            neg2 = sb.tile([128, 1], F32, tag="neg2")
            nc.vector.memset(neg2[:], -2.0)
            for j in range(OT):
                bp = bps.tile([128, 8], F32, tag="bps")
                for a in range(IT):
                    nc.tensor.matmul(
                        bp[:],
                        lhsT=wv_sb[0][:, a, j * 128 : (j + 1) * 128],
                        rhs=ones[:],
                        start=(a == 0),
                        stop=(a == IT - 1),
                    )
                nc.scalar.activation(bias_sb[:, j : j + 1], bp[:, 0:1], AF.Copy)

            flat = [128, IT * CHUNK]

            def emit_evictions(pending):
                # Evict the previous chunk's PSUM groups. Emitted AFTER the
                # next chunk's basis DAG so the strict-FIFO ACT queue
                # prioritizes producing the basis the PE is waiting on.
                for c0, j, acc in pending:
                    bsl0 = slice(c0 * CHUNK, (c0 + 1) * CHUNK)
                    o_sb = outp.tile([128, CHUNK], F32, tag="out")
                    nc.scalar.activation(
                        o_sb[:], acc[:], AF.Identity, bias=bias_sb[:, j : j + 1]
                    )
                    nc.sync.dma_start(
                        out=yt[j * 128 : (j + 1) * 128, bsl0], in_=o_sb[:]
                    )

            pending = []
            for c in range(N_CHUNKS):
                bsl = slice(c * CHUNK, (c + 1) * CHUNK)
                if c == 0:
                    x_sb = x0_sb
                else:
                    x_sb = xtp.tile([128, IT, CHUNK], F32, tag="xt")
                    nc.sync.dma_start(
                        out=x_sb[:],
                        in_=xt[:, bsl].rearrange("(a p) b -> p a b", p=128),
                    )
                xf = x_sb[:].rearrange("p a b -> p (a b)")

                # --- fp32 basis DAG (exact U_d up to scale/shift) ---
                # t = tanh x;  s4 = 4t^2 = U2+1;  b3 = (s4-2)t = U3/2
                # q2 = (s4-1)^2 = U2^2;  b4 = q2-s4 = U4
                # b5 = (s4-2)b3 = (U5+2t)/2;  q3 = b3^2
                # b6 = 4q3-q2 = U6;  e4 = (s4-2)^2;  b7 = (e4-2)b3 = U7/2
                # (f32r boundary copies right after each producer so the
                # first matmuls of the chunk can start as early as possible)
                t = sb.tile(flat, F32, tag="t")
                nc.scalar.activation(t[:], xf, AF.Tanh)
                t_r = sb.tile(flat, F32R, tag="t_r")
                nc.vector.tensor_copy(t_r[:], t[:])
                s4 = sb.tile(flat, F32, tag="s4")
                nc.scalar.activation(s4[:], t[:], AF.Square, scale=2.0)
                s4_r = sb.tile(flat, F32R, tag="s4_r")
                nc.vector.tensor_copy(s4_r[:], s4[:])
                b3 = sb.tile(flat, F32, tag="b3")
                nc.vector.scalar_tensor_tensor(
                    b3[:], s4[:], 2.0, t[:], ALU.subtract, ALU.mult
                )
                b3_r = sb.tile(flat, F32R, tag="b3_r")
                nc.vector.tensor_copy(b3_r[:], b3[:])
                q2 = sb.tile(flat, F32, tag="q2")
                nc.scalar.activation(q2[:], s4[:], AF.Square, bias=neg1[:])
                b4 = sb.tile(flat, F32, tag="b4")
                nc.vector.tensor_sub(b4[:], q2[:], s4[:])
                b4_r = sb.tile(flat, F32R, tag="b4_r")
                nc.vector.tensor_copy(b4_r[:], b4[:])
                b5 = sb.tile(flat, F32R, tag="b5")
                nc.vector.scalar_tensor_tensor(
                    b5[:], s4[:], 2.0, b3[:], ALU.subtract, ALU.mult
                )
                q3 = sb.tile(flat, F32, tag="q3")
                nc.scalar.activation(q3[:], b3[:], AF.Square)
                b6 = sb.tile(flat, F32R, tag="b6")
                nc.vector.scalar_tensor_tensor(
                    b6[:], q3[:], 4.0, q2[:], ALU.mult, ALU.subtract
                )
                e4 = sb.tile(flat, F32, tag="e4")
                nc.scalar.activation(e4[:], s4[:], AF.Square, bias=neg2[:])
                b7 = sb.tile(flat, F32R, tag="b7")
                nc.vector.scalar_tensor_tensor(
                    b7[:], e4[:], 2.0, b3[:], ALU.subtract, ALU.mult
                )

                basis = [t_r, s4_r, b3_r, b4_r, b5, b6, b7]  # k = 1..7

                if pending:
                    emit_evictions(pending)
                    pending = []

                for j in range(OT):
                    acc = ps.tile([128, CHUNK], F32, tag="acc")
                    n_mm = (DEG - 1) * IT
                    idx = 0
                    for k in range(1, DEG):
                        pk = basis[k - 1]
                        for a in range(IT):
                            nc.tensor.matmul(
                                acc[:],
                                lhsT=wv_sb[k][:, a, j * 128 : (j + 1) * 128],
                                rhs=pk[:, a * CHUNK : (a + 1) * CHUNK],
                                start=(idx == 0),
                                stop=(idx == n_mm - 1),
                            )
                            idx += 1
                    pending.append((c, j, acc))

            emit_evictions(pending)

    nc.compile()
    return nc


_NC_CACHE = None
_last_in_maps = None


def _get_nc():
    global _NC_CACHE
    if _NC_CACHE is None:
        _NC_CACHE = _build_nc()
    return _NC_CACHE


def kernel(x: np.ndarray, gegenbauer_coeffs: np.ndarray, **unused) -> np.ndarray:
    x = np.asarray(x, dtype=np.float32).reshape(B, I)
    coeffs = np.asarray(gegenbauer_coeffs, dtype=np.float32)

    # Host prep: basis change (exact integers, applied in fp64) and layouts.
    M = _basis_matrix()
    v = np.einsum("iod,dk->kio", coeffs.astype(np.float64), M)
    wv = np.ascontiguousarray(v.reshape(DEG * I, O).astype(np.float32))
    xt_full = np.ascontiguousarray(x.T)  # [I, B]

    in_maps = []
    for c in range(N_CORES):
        xt_c = np.ascontiguousarray(xt_full[:, c * B_LOC : (c + 1) * B_LOC])
        in_maps.append({"xt": xt_c, "wv": wv})

    global _last_in_maps
    _last_in_maps = in_maps

    nc = _get_nc()
    try:
        res = run_bass_kernel_spmd(nc, in_maps, core_ids=list(range(N_CORES)))
    except Exception:
        # A previous crashed session can leave a core unrecoverable until
        # the runtime resets it; one retry clears it.
        res = run_bass_kernel_spmd(nc, in_maps, core_ids=list(range(N_CORES)))

    y = np.empty((B, O), dtype=np.float32)
    for c in range(N_CORES):
        y[c * B_LOC : (c + 1) * B_LOC, :] = res.results[c]["yt"].T
    return y


# revision 21
# speedup vs baseline: 1.8053x; 1.0623x over previous
"""GegenbauerKAN layer (alpha=1 -> Chebyshev-U basis) on 8 TRN2 NeuronCores.

Math: y[b,o] = sum_{i,d} C_d(tanh(x[b,i])) * W[i,o,d],  d=0..7,
where C_d are Gegenbauer(alpha=1) = Chebyshev-U polynomials.

Strategy:
  - Data-parallel over batch: each of the 8 cores handles 2048 rows.
  - Transposed layout: the host feeds x^T slices so the contraction
    index i lives on SBUF partitions with no on-device transposes.
  - On-device basis: exact U_d values via Chebyshev addition formulas
    (U_{m+n} = U_m U_n - U_{m-1} U_{n-1}), all in fp32:
        t  = tanh(x)            s4 = (2t)^2 = U2+1
        b3 = (s4-2)t = U3/2     q2 = (s4-1)^2 = U2^2
        b4 = q2-s4   = U4       d  = b3-t    = (U3-U1)/2
        b5 = (s4-1)d = U5/2     q3 = b3^2    = U3^2/4
        b6 = 4q3-q2  = U6       e  = b4+1-s4 = U4-U2
        b7 = e*b3    = U7/2
    This is numerically identical in conditioning to the textbook
    recurrence but needs only 7 DVE + 4 ACT ops per chunk.
  - Matmuls run in float32r (TRN2 fast fp32 PE path, 1 cycle/row for
    moving dim >= 256, ~= 11-bit-mantissa rounding of the operands).
    Basis tensors and weights are rounded to f32r exactly once, at the
    matmul boundary; all basis arithmetic stays fp32. Measured/simulated
    end-to-end max error ~1.7e-4 * absmax(y) (fp32 ref itself: ~5e-6).
  - k=0 (U_0 = 1) is folded into a per-output bias computed on-device
    from the k=0 weight block with ones-vector matmuls, added at PSUM
    eviction (saves 1/8 of the matmul work).
  - Weights basis change on host: y = sum_k phi_k . V_k with
    V[:,:,k] = sum_d W[:,:,d] M[d,k], M the (exact, tiny) change of
    basis from {phi_k} to {U_d}; applied in fp64, rounded once.
"""

import numpy as np

import concourse.bacc as bacc
import concourse.mybir as mybir
import concourse.tile as tile
from concourse.alu_op_type import AluOpType as ALU
from concourse.bass_utils import run_bass_kernel_spmd

F32 = mybir.dt.float32
F32R = mybir.dt.float32r
AF = mybir.ActivationFunctionType

N_CORES = 8
B = 16384
I = 512
O = 512
DEG = 8  # degrees 0..7
B_LOC = B // N_CORES  # 2048 rows per core
CHUNK = 512  # b columns processed per pipeline stage
N_CHUNKS = B_LOC // CHUNK
IT = I // 128  # 4 partition tiles of the input-feature dim
OT = O // 128  # 4 partition tiles of the output dim


def _basis_matrix() -> np.ndarray:
    """M[d,k]: U_d = sum_k M[d,k] * phi_k for the on-device basis
    phi = [1, t, s4, b3, b4, b5, b6, b7]."""
    M = np.zeros((DEG, DEG))
    M[0, 0] = 1.0
    M[1, 1] = 2.0  # U1 = 2 t
    M[2, 0] = -1.0
    M[2, 2] = 1.0  # U2 = s4 - 1
    M[3, 3] = 2.0  # U3 = 2 b3
    M[4, 4] = 1.0  # U4 = b4
    M[5, 5] = 2.0
    M[5, 1] = -2.0  # U5 = 2 b5 - 2 t
    M[6, 6] = 1.0  # U6 = b6
    M[7, 7] = 2.0  # U7 = 2 b7
    return M


def _build_nc():
    nc = bacc.Bacc("TRN2", target_bir_lowering=False, debug=False)

    xt = nc.dram_tensor("xt", [I, B_LOC], F32, kind="ExternalInput")
    # f32r end-to-end: the PE rounds matmul operands to f32r precision
    # regardless; typing the DRAM tensor f32r lets the fast HWDGE (sync)
    # engine load it with no cast.
    wv = nc.dram_tensor("wv", [DEG * I, O], F32R, kind="ExternalInput")
    yt = nc.dram_tensor("yt", [O, B_LOC], F32, kind="ExternalOutput")

    with tile.TileContext(nc) as tc:
        with (
            tc.tile_pool(name="wvp", bufs=1) as wvp,
            tc.tile_pool(name="sb", bufs=1) as sb,
            tc.tile_pool(name="xtp", bufs=1) as xtp,
            tc.tile_pool(name="outp", bufs=4) as outp,
            tc.tile_pool(name="ps", bufs=6, space="PSUM") as ps,
            tc.tile_pool(name="bps", bufs=2, space="PSUM") as bps,
        ):
            # Chunk 0's x slice first on the sync queue so tanh can start
            # as early as possible.
            x0_sb = xtp.tile([128, IT, CHUNK], F32, tag="xt")
            nc.sync.dma_start(
                out=x0_sb[:],
                in_=xt[:, 0:CHUNK].rearrange("(a p) b -> p a b", p=128),
            )

            # Weights: 8 tiles [128, IT, 512] (k-major rows), split across
            # the gpsimd and sync DMA queues, ordered by when the k-major
            # matmul loop first needs them.
            wv_sb = [None] * DEG
            for k in [1, 2, 3, 4, 5, 6, 7, 0]:
                w = wvp.tile([128, IT, O], F32R, tag=f"wv{k}")
                nc.gpsimd.dma_start(
                    out=w[:],
                    in_=wv[k * I : (k + 1) * I, :].rearrange(
                        "(a p) o -> p a o", p=128
                    ),
                )
                wv_sb[k] = w

            # Per-output bias from the k=0 block: bias[o] = sum_i V[i,o,0].
            ones_f = sb.tile([128, 8], F32, tag="ones_f")
            nc.vector.memset(ones_f[:], 1.0)
            ones = sb.tile([128, 8], F32R, tag="ones")
            nc.vector.tensor_copy(ones[:], ones_f[:])
            bias_sb = sb.tile([128, OT], F32, tag="bias")
            neg1 = sb.tile([128, 1], F32, tag="neg1")
            nc.vector.memset(neg1[:], -1.0)
            neg2 = sb.tile([128, 1], F32, tag="neg2")
            nc.vector.memset(neg2[:], -2.0)
            for j in range(OT):
                bp = bps.tile([128, 8], F32, tag="bps")
                for a in range(IT):
                    nc.tensor.matmul(
                        bp[:],
                        lhsT=wv_sb[0][:, a, j * 128 : (j + 1) * 128],
                        rhs=ones[:],
                        start=(a == 0),
                        stop=(a == IT - 1),
                    )
                nc.scalar.activation(bias_sb[:, j : j + 1], bp[:, 0:1], AF.Copy)

            flat = [128, IT * CHUNK]

            def emit_evictions(pending):
                # Evict the previous chunk's PSUM groups. Emitted AFTER the
                # next chunk's basis DAG so the strict-FIFO ACT queue
                # prioritizes producing the basis the PE is waiting on.
                for c0, j, acc in pending:
                    bsl0 = slice(c0 * CHUNK, (c0 + 1) * CHUNK)
                    o_sb = outp.tile([128, CHUNK], F32, tag="out")
                    nc.scalar.activation(
                        o_sb[:], acc[:], AF.Identity, bias=bias_sb[:, j : j + 1]
                    )
                    nc.sync.dma_start(
                        out=yt[j * 128 : (j + 1) * 128, bsl0], in_=o_sb[:]
                    )

            pending = []
            for c in range(N_CHUNKS):
                bsl = slice(c * CHUNK, (c + 1) * CHUNK)
                if c == 0:
                    x_sb = x0_sb
                else:
                    x_sb = xtp.tile([128, IT, CHUNK], F32, tag="xt")
                    nc.sync.dma_start(
                        out=x_sb[:],
                        in_=xt[:, bsl].rearrange("(a p) b -> p a b", p=128),
                    )
                xf = x_sb[:].rearrange("p a b -> p (a b)")

                # --- fp32 basis DAG (exact U_d up to scale/shift) ---
                # t = tanh x;  s4 = 4t^2 = U2+1;  b3 = (s4-2)t = U3/2
                # q2 = (s4-1)^2 = U2^2;  b4 = q2-s4 = U4
                # b5 = (s4-2)b3 = (U5+2t)/2;  q3 = b3^2
                # b6 = 4q3-q2 = U6;  e4 = (s4-2)^2;  b7 = (e4-2)b3 = U7/2
                # (f32r boundary copies right after each producer so the
                # first matmuls of the chunk can start as early as possible)
                t = sb.tile(flat, F32, tag="t")
                nc.scalar.activation(t[:], xf, AF.Tanh)
                t_r = sb.tile(flat, F32R, tag="t_r")
                nc.vector.tensor_copy(t_r[:], t[:])
                s4 = sb.tile(flat, F32, tag="s4")
                nc.scalar.activation(s4[:], t[:], AF.Square, scale=2.0)
                s4_r = sb.tile(flat, F32R, tag="s4_r")
                nc.scalar.activation(s4_r[:], s4[:], AF.Identity)
                b3 = sb.tile(flat, F32, tag="b3")
                nc.vector.scalar_tensor_tensor(
                    b3[:], s4[:], 2.0, t[:], ALU.subtract, ALU.mult
                )
                b3_r = sb.tile(flat, F32R, tag="b3_r")
                nc.vector.tensor_copy(b3_r[:], b3[:])
                q2 = sb.tile(flat, F32, tag="q2")
                nc.scalar.activation(q2[:], s4[:], AF.Square, bias=neg1[:])
                b4 = sb.tile(flat, F32, tag="b4")
                nc.vector.tensor_sub(b4[:], q2[:], s4[:])
                b4_r = sb.tile(flat, F32R, tag="b4_r")
                nc.vector.tensor_copy(b4_r[:], b4[:])
                b5 = sb.tile(flat, F32R, tag="b5")
                nc.vector.scalar_tensor_tensor(
                    b5[:], s4[:], 2.0, b3[:], ALU.subtract, ALU.mult
                )
                q3 = sb.tile(flat, F32, tag="qe")
                nc.scalar.activation(q3[:], b3[:], AF.Square)
                b6 = sb.tile(flat, F32R, tag="b6")
                nc.vector.scalar_tensor_tensor(
                    b6[:], q3[:], 4.0, q2[:], ALU.mult, ALU.subtract
                )
                e4 = sb.tile(flat, F32, tag="qe")
                nc.scalar.activation(e4[:], s4[:], AF.Square, bias=neg2[:])
                b7 = sb.tile(flat, F32R, tag="b7")
                nc.vector.scalar_tensor_tensor(
                    b7[:], e4[:], 2.0, b3[:], ALU.subtract, ALU.mult
                )

                basis = [t_r, s4_r, b3_r, b4_r, b5, b6, b7]  # k = 1..7

                if pending:
                    emit_evictions(pending)
                    pending = []

                for j in range(OT):
                    acc = ps.tile([128, CHUNK], F32, tag="acc")
                    n_mm = (DEG - 1) * IT
                    idx = 0
                    for k in range(1, DEG):
                        pk = basis[k - 1]
                        for a in range(IT):
                            nc.tensor.matmul(
                                acc[:],
                                lhsT=wv_sb[k][:, a, j * 128 : (j + 1) * 128],
                                rhs=pk[:, a * CHUNK : (a + 1) * CHUNK],
                                start=(idx == 0),
                                stop=(idx == n_mm - 1),
                            )
                            idx += 1
                    pending.append((c, j, acc))

            emit_evictions(pending)

    nc.compile()
    return nc


_NC_CACHE = None
_last_in_maps = None


def _get_nc():
    global _NC_CACHE
    if _NC_CACHE is None:
        _NC_CACHE = _build_nc()
    return _NC_CACHE


def kernel(x: np.ndarray, gegenbauer_coeffs: np.ndarray, **unused) -> np.ndarray:
    x = np.asarray(x, dtype=np.float32).reshape(B, I)
    coeffs = np.asarray(gegenbauer_coeffs, dtype=np.float32)

    # Host prep: basis change (exact integers, applied in fp64) and layouts.
    M = _basis_matrix()
    v = np.einsum("iod,dk->kio", coeffs.astype(np.float64), M)
    wv = np.ascontiguousarray(v.reshape(DEG * I, O).astype(np.float32))
    xt_full = np.ascontiguousarray(x.T)  # [I, B]

    in_maps = []
    for c in range(N_CORES):
        xt_c = np.ascontiguousarray(xt_full[:, c * B_LOC : (c + 1) * B_LOC])
        in_maps.append({"xt": xt_c, "wv": wv})

    global _last_in_maps
    _last_in_maps = in_maps

    nc = _get_nc()
    try:
        res = run_bass_kernel_spmd(nc, in_maps, core_ids=list(range(N_CORES)))
    except Exception:
        # A previous crashed session can leave a core unrecoverable until
        # the runtime resets it; one retry clears it.
        res = run_bass_kernel_spmd(nc, in_maps, core_ids=list(range(N_CORES)))

    y = np.empty((B, O), dtype=np.float32)
    for c in range(N_CORES):
        y[c * B_LOC : (c + 1) * B_LOC, :] = res.results[c]["yt"].T
    return y
